# revision 14
# baseline (speedup 1.0000x reference)
"""Multi-head GAT layer (2 heads, sum-merged) on 8 TRN2 NeuronCores.

Strategy: edges sharded by destination node (12500 dsts per core); the
segment softmax and scatter-sum are core-local (no collectives). Node
features and weights replicated; every core computes the projected source
table Z = [z | s_src] (bf16) once, then processes its edges in fixed
128-dst windows via indirect-DMA row gathers. All data-dependent
structure is carried in index tensors so the compiled program is
identical across cores (SPMD); per-window gather-column counts (K_t) are
taken as the max over cores so the program is shared.

Fixed windows make the s_dst window load and the output write static
direct DMAs (HWDGE) — the only pool-engine (SWDGE) work is the per-128-
edge z-row gathers, which are the hard throughput wall (~1.3us each).
"""

import numpy as np
import ml_dtypes

import concourse.bass as bass
import concourse.bacc as bacc
import concourse.mybir as mybir
import concourse.tile as tile
from concourse.bass_utils import run_bass_kernel_spmd

F32 = mybir.dt.float32
BF16 = mybir.dt.bfloat16
I32 = mybir.dt.int32

IN = 128          # input feature dim
OUT = 64          # output feature dim per head
H = 2             # heads
ZC = IN + 2       # z-row: 128 z + 2 s_src (all bf16)
NCORES = 8

N_SRC = 100000
N_DST = 100000
NDST_C = N_DST // NCORES            # 12500 dsts per core
SRC_TILES = 784                     # 784*128 = 100352 >= N_SRC
SRC_PAD = SRC_TILES * 128
SRC_GROUP = 16                      # src tiles per load group (49 groups)
DST_TILES = 98                      # 98*128 = 12544 >= NDST_C
DST_PAD = DST_TILES * 128
DST_GROUP = 7                       # dst tiles per load group (14 groups)
T_WIN = DST_TILES                   # 98 fixed 128-dst windows per core


def _perm_row(n):
    """Z-table row for src node n (layout: contiguous per partition)."""
    g = n // (SRC_GROUP * 128)
    m = n % (SRC_GROUP * 128)
    return g * (SRC_GROUP * 128) + (m % 128) * SRC_GROUP + (m // 128)


def _pack_all(src_idx, dst_idx):
    """Bucket edges into (core, 128-dst window); emit per-window slot
    tensors. Returns eidx [NCORES, T_WIN, 128, 2*KMAX] and K list."""
    core_of = dst_idx // NDST_C
    counts = np.zeros((NCORES, T_WIN), np.int64)
    buckets = [[None] * T_WIN for _ in range(NCORES)]
    for c in range(NCORES):
        m = core_of == c
        s_c = src_idx[m]
        d_c = dst_idx[m] - c * NDST_C
        w_c = d_c // 128
        order = np.argsort(w_c, kind="stable")
        s_c, d_c, w_c = s_c[order], d_c[order], w_c[order]
        starts = np.searchsorted(w_c, np.arange(T_WIN + 1))
        for t in range(T_WIN):
            lo, hi = starts[t], starts[t + 1]
            buckets[c][t] = (s_c[lo:hi], d_c[lo:hi] - t * 128)
            counts[c, t] = hi - lo
    kt = np.maximum(1, -(-counts.max(axis=0) // 128)).astype(np.int64)
    kmax = int(kt.max())
    eidx = np.zeros((NCORES, T_WIN, 128, 2 * kmax), np.int32)
    eidx[:, :, :, kmax:] = -1          # dstrel pad
    rp = _perm_row(np.arange(SRC_PAD))
    GRP = SRC_GROUP * 128
    rmax = np.zeros((T_WIN, kmax), np.int64)
    for c in range(NCORES):
        for t in range(T_WIN):
            s, drel = buckets[c][t]
            g = rp[s]
            o2 = np.argsort(g, kind="stable")   # columns sorted by Z row
            g, drel = g[o2], drel[o2]
            n = len(g)
            e = np.arange(n)
            p, j = e % 128, e // 128
            eidx[c, t, p, j] = g
            eidx[c, t, p, kmax + j] = drel
            for jj in range(kt[t]):
                hi = min((jj + 1) * 128, n)
                if hi > 0:
                    rmax[t, jj] = max(rmax[t, jj], g[min(hi, n) - 1] + 1)
    # round dependency prefix up to whole phase-A write groups
    r_cols = (np.maximum(1, -(-rmax // GRP)) * GRP).clip(max=SRC_PAD)
    return eidx, [int(k) for k in kt], kmax, r_cols.astype(np.int64)


NQ = 4            # SWDGE queues to spread indirect gathers across


def _build_program(K_list, KMAX, R_cols):
    nc = bacc.Bacc("TRN2", target_bir_lowering=False, debug=False,
                   num_devices=NCORES, num_swdge_queues=NQ)
    hsT = nc.dram_tensor("hsrcT", [128, SRC_PAD], BF16, kind="ExternalInput").ap()
    hdT = nc.dram_tensor("hdstT", [128, DST_PAD], BF16, kind="ExternalInput").ap()
    wsr = nc.dram_tensor("wsrc", [128, ZC], BF16, kind="ExternalInput").ap()
    wds = nc.dram_tensor("wdst", [128, 2], BF16, kind="ExternalInput").ap()
    eix = nc.dram_tensor("eidx", [T_WIN, 128, 2 * KMAX], I32,
                         kind="ExternalInput").ap()
    Z = nc.dram_tensor("Z", [SRC_PAD, ZC], BF16, kind="Internal").ap()
    SD = nc.dram_tensor("SD", [DST_PAD, 2], BF16, kind="Internal").ap()
    out = nc.dram_tensor("out", [DST_PAD, OUT], F32, kind="ExternalOutput").ap()

    from concourse.masks import make_identity

    with tile.TileContext(nc) as tc:
        with (
            tc.tile_pool(name="const", bufs=1) as cpool,
            tc.tile_pool(name="pa", bufs=3) as pa_pool,
            tc.tile_pool(name="pz", bufs=3) as pz_pool,
            tc.tile_pool(name="sda", bufs=1) as sd_pool,
            tc.tile_pool(name="ei", bufs=12) as ei_pool,
            tc.tile_pool(name="zg", bufs=11) as zg_pool,
            tc.tile_pool(name="oht", bufs=5) as oht_pool,
            tc.tile_pool(name="wt", bufs=6) as w_pool,
            tc.tile_pool(name="fl", bufs=6) as f_pool,
        ):
            wsrc_t = cpool.tile([128, ZC], BF16)
            nc.sync.dma_start(out=wsrc_t[:], in_=wsr[:, :])
            wdst_t = cpool.tile([128, 2], BF16)
            nc.sync.dma_start(out=wdst_t[:], in_=wds[:, :])
            iota_t = cpool.tile([128, KMAX * 128], I32)
            nc.gpsimd.iota(iota_t[:], [[0, KMAX], [1, 128]], channel_multiplier=0)
            iop_t = cpool.tile([128, 1], BF16)
            nc.gpsimd.iota(iop_t[:], [[0, 1]], channel_multiplier=1,
                           allow_small_or_imprecise_dtypes=True)
            ident_t = cpool.tile([128, 128], BF16)
            make_identity(nc, ident_t[:])

            # ---- Phase A: s_dst first (unblocks phase-B sd machinery) ----
            with (
                tc.tile_pool(name="psA", bufs=4, space="PSUM") as psA_pool,
                tc.tile_pool(name="psD", bufs=2, space="PSUM") as psD_pool,
            ):
                sdall = sd_pool.tile([128, 2 * DST_TILES], BF16)
                for g in range(DST_TILES // DST_GROUP):
                    hTd = pa_pool.tile([128, DST_GROUP * 128], BF16, tag="hTd")
                    nc.scalar.dma_start(
                        out=hTd[:],
                        in_=hdT[:, g * DST_GROUP * 128:(g + 1) * DST_GROUP * 128])
                    for j in range(DST_GROUP):
                        t = g * DST_GROUP + j
                        psd = psD_pool.tile([128, 2], F32, tag="psd")
                        nc.tensor.matmul(
                            out=psd[:], lhsT=hTd[:, j * 128:(j + 1) * 128],
                            rhs=wdst_t[:], start=True, stop=True)
                        nc.any.tensor_copy(
                            out=sdall[:, t * 2:(t + 1) * 2], in_=psd[:])
                nc.sync.dma_start(
                    out=SD[:, :].rearrange("(t p) c -> p t c", p=128),
                    in_=sdall[:].rearrange("p (t c) -> p t c", c=2))

                # ---- Phase A: Z = [z | s_src] bf16 for all src nodes ----
                for g in range(SRC_TILES // SRC_GROUP):
                    hT = pa_pool.tile([128, SRC_GROUP * 128], BF16)
                    nc.scalar.dma_start(
                        out=hT[:],
                        in_=hsT[:, g * SRC_GROUP * 128:(g + 1) * SRC_GROUP * 128])
                    zbig = pz_pool.tile([128, SRC_GROUP * ZC], BF16)
                    for m in range(SRC_GROUP // 2):
                        ps = psA_pool.tile([128, 2 * ZC], F32)
                        for h in range(2):
                            nc.tensor.matmul(
                                out=ps[:, h * ZC:(h + 1) * ZC],
                                lhsT=hT[:, (2 * m + h) * 128:(2 * m + h + 1) * 128],
                                rhs=wsrc_t[:], start=True, stop=True)
                        nc.any.tensor_copy(
                            out=zbig[:, m * 2 * ZC:(m + 1) * 2 * ZC], in_=ps[:])
                    rows = slice(g * SRC_GROUP * 128, (g + 1) * SRC_GROUP * 128)
                    # permuted layout: row g*2048 + p*16 + j <- zbig[p, j]
                    nc.sync.dma_start(
                        out=Z[rows, :].rearrange("(p j) c -> p j c", p=128),
                        in_=zbig[:].rearrange("p (j c) -> p j c", c=ZC))

            # ---- Phase B: fixed 128-dst windows ----
            with (
                tc.tile_pool(name="psB", bufs=4, space="PSUM") as psB_pool,
                tc.tile_pool(name="psOH", bufs=2, space="PSUM") as psOH_pool,
                tc.tile_pool(name="psSD", bufs=2, space="PSUM") as psSD_pool,
            ):
              for t in range(T_WIN):
                  K = K_list[t]
                  NB = (K + 7) // 8          # psOH banks (8 transposes each)
                  ei = ei_pool.tile([128, 2 * KMAX], I32)
                  nc.scalar.dma_start(out=ei[:], in_=eix[t, :, :])
                  zg = zg_pool.tile([128, KMAX * ZC], BF16)
                  for j in range(K):
                      # columns are src-sorted: this gather only reads the
                      # Z prefix [0, R) -> overlaps with phase-A tail writes
                      R = int(R_cols[t][j])
                      nc.gpsimd.indirect_dma_start(
                          out=zg[:, j * ZC:(j + 1) * ZC], out_offset=None,
                          in_=Z[0:R, :],
                          in_offset=bass.IndirectOffsetOnAxis(
                              ap=ei[:, j:j + 1], axis=0))
                  zg3 = zg[:].rearrange("p (j c) -> p j c", c=ZC)
                  sdw = w_pool.tile([128, 2], BF16, tag="sdw")
                  nc.scalar.dma_start(out=sdw[:], in_=SD[t * 128:(t + 1) * 128, :])
                  drelF = w_pool.tile([128, KMAX], BF16, tag="drelF")
                  nc.vector.tensor_copy(out=drelF[:, 0:K],
                                        in_=ei[:, KMAX:KMAX + K])
                  # transposed one-hots (slot-dst -> partition), 8 per bank
                  ohs = oht_pool.tile([128, KMAX * 128], BF16, tag="ohs")
                  ps_sd = psSD_pool.tile([128, 2 * KMAX], F32, tag="pssd")
                  for b in range(NB):
                      nb = min(8, K - b * 8)
                      ps_oh = psOH_pool.tile([128, 8 * 128], BF16, tag="psoh")
                      for i in range(nb):
                          j = b * 8 + i
                          nc.tensor.transpose(
                              out=ps_oh[:, i * 128:(i + 1) * 128],
                              in_=drelF[:, j:j + 1].to_broadcast([128, 128]),
                              identity=ident_t[:])
                      nc.vector.tensor_tensor(
                          out=ohs[:, b * 1024:b * 1024 + nb * 128],
                          in0=iop_t[:, 0:1].to_broadcast([128, nb * 128]),
                          in1=ps_oh[:, 0:nb * 128],
                          op=mybir.AluOpType.is_equal)
                  for j in range(K):
                      nc.tensor.matmul(
                          out=ps_sd[:, j * 2:(j + 1) * 2],
                          lhsT=ohs[:, j * 128:(j + 1) * 128],
                          rhs=sdw[:], start=True, stop=True)
                  oht = oht_pool.tile([128, KMAX * 128], BF16)
                  nc.vector.tensor_tensor(
                      out=oht[:, 0:K * 128],
                      in0=ei[:, KMAX:KMAX + K].to_broadcast([128, K, 128]),
                      in1=iota_t[:, 0:K * 128].rearrange(
                          "p (k q) -> p k q", q=128),
                      op=mybir.AluOpType.is_equal)
                  st = w_pool.tile([128, KMAX * 2], F32, tag="st")
                  nc.vector.tensor_tensor(
                      out=st[:, 0:K * 2].rearrange("p (j c) -> p j c", c=2),
                      in0=zg3[:, 0:K, IN:IN + 2],
                      in1=ps_sd[:, 0:K * 2].rearrange("p (j c) -> p j c", c=2),
                      op=mybir.AluOpType.add)
                  st2 = w_pool.tile([128, KMAX * 2], F32, tag="st2")
                  nc.vector.tensor_scalar_mul(out=st2[:, 0:K * 2],
                                              in0=st[:, 0:K * 2], scalar1=0.01)
                  nc.vector.tensor_tensor(out=st[:, 0:K * 2], in0=st[:, 0:K * 2],
                                          in1=st2[:, 0:K * 2],
                                          op=mybir.AluOpType.max)
                  wt = w_pool.tile([128, KMAX * 2], F32, tag="wt")
                  nc.scalar.activation(
                      out=wt[:, 0:K * 2], in_=st[:, 0:K * 2],
                      func=mybir.ActivationFunctionType.Exp)
                  wt3 = wt[:].rearrange("p (j c) -> p j c", c=2)
                  wzb = zg_pool.tile([128, KMAX * ZC], BF16, tag="wzb")
                  wzb3 = wzb[:].rearrange("p (j c) -> p j c", c=ZC)
                  for h in range(H):
                      nc.vector.tensor_tensor(
                          out=wzb3[:, 0:K, h * OUT:(h + 1) * OUT],
                          in0=zg3[:, 0:K, h * OUT:(h + 1) * OUT],
                          in1=wt3[:, 0:K, h:h + 1].to_broadcast([128, K, OUT]),
                          op=mybir.AluOpType.mult)
                  nc.vector.tensor_copy(out=wzb3[:, 0:K, IN:IN + 2],
                                        in_=wt3[:, 0:K, :])
                  ps = psB_pool.tile([128, ZC], F32)
                  for j in range(K):
                      nc.tensor.matmul(
                          out=ps[:],
                          lhsT=oht[:, j * 128:(j + 1) * 128],
                          rhs=wzb[:, j * ZC:(j + 1) * ZC],
                          start=(j == 0), stop=(j == K - 1))
                  den = f_pool.tile([128, 2], F32, tag="den")
                  nc.vector.tensor_scalar_max(
                      out=den[:], in0=ps[:, IN:IN + 2], scalar1=1e-30)
                  rec = f_pool.tile([128, 2], F32, tag="rec")
                  nc.vector.reciprocal(out=rec[:], in_=den[:])
                  o0 = f_pool.tile([128, OUT], F32, tag="o0")
                  nc.vector.tensor_scalar_mul(
                      out=o0[:], in0=ps[:, 0:OUT], scalar1=rec[:, 0:1])
                  ot = f_pool.tile([128, OUT], F32, tag="ot")
                  nc.vector.tensor_scalar_mul(
                      out=ot[:], in0=ps[:, OUT:2 * OUT], scalar1=rec[:, 1:2])
                  nc.vector.tensor_add(out=ot[:], in0=ot[:], in1=o0[:])
                  nc.sync.dma_start(out=out[t * 128:(t + 1) * 128, :], in_=ot[:])

    nc.compile()
    return nc


def _prep_inputs(h_src, h_dst, W_src, W_dst, a_w, src_idx, dst_idx):
    """Host-side sharding/layout prep. Returns in_maps for the 8 cores."""
    hs = np.zeros((SRC_PAD, IN), np.float32)
    hs[:N_SRC] = h_src
    hsrcT = np.ascontiguousarray(hs.T.astype(ml_dtypes.bfloat16))

    # wsrc: [IN, ZC] = [ W[h,o,d] at col h*OUT+o | w~_s ]
    wsr = np.zeros((IN, ZC), np.float32)
    wsr[:, :H * OUT] = W_src.reshape(H * OUT, IN).T
    a_s, a_d = a_w[:, :OUT], a_w[:, OUT:]
    wsr[:, H * OUT:H * OUT + H] = np.einsum("hod,ho->dh", W_src, a_s)
    wsr = wsr.astype(ml_dtypes.bfloat16)
    wds = np.einsum("hod,ho->dh", W_dst, a_d).astype(ml_dtypes.bfloat16)

    eidx, K_list, KMAX, R_cols = _pack_all(src_idx, dst_idx)

    in_maps = []
    for c in range(NCORES):
        hd = np.zeros((DST_PAD, IN), np.float32)
        hd[:NDST_C] = h_dst[c * NDST_C:(c + 1) * NDST_C]
        hdstT = np.ascontiguousarray(hd.T.astype(ml_dtypes.bfloat16))
        in_maps.append({
            "hsrcT": hsrcT,
            "hdstT": hdstT,
            "wsrc": wsr,
            "wdst": wds,
            "eidx": eidx[c],
        })
    return in_maps, K_list, KMAX, R_cols


def _run(inputs, trace=False):
    inputs = {k: np.asarray(v) for k, v in inputs.items()}
    in_maps, K_list, KMAX, R_cols = _prep_inputs(**inputs)
    nc = _build_program(K_list, KMAX, R_cols)
    res = run_bass_kernel_spmd(
        nc, in_maps, core_ids=list(range(NCORES)), trace=trace)
    parts = [res.results[c]["out"][:NDST_C] for c in range(NCORES)]
    return np.concatenate(parts, axis=0), res


def kernel(**inputs):
    out, _ = _run(inputs, trace=False)
    return out


# revision 15
# speedup vs baseline: 1.0673x; 1.0673x over previous
"""Multi-head GAT layer (2 heads, sum-merged) on 8 TRN2 NeuronCores.

Strategy: edges sharded by destination node (12500 dsts per core); the
segment softmax and scatter-sum are core-local (no collectives). Node
features and weights replicated; every core computes the projected source
table Z = [z | s_src] (bf16) once, then processes its edges in fixed
128-dst windows via indirect-DMA row gathers. All data-dependent
structure is carried in index tensors so the compiled program is
identical across cores (SPMD); per-window gather-column counts (K_t) are
taken as the max over cores so the program is shared.

Fixed windows make the s_dst window load and the output write static
direct DMAs (HWDGE) — the only pool-engine (SWDGE) work is the per-128-
edge z-row gathers, which are the hard throughput wall (~1.3us each).
"""

import numpy as np
import ml_dtypes

import concourse.bass as bass
import concourse.bacc as bacc
import concourse.mybir as mybir
import concourse.tile as tile
from concourse.bass_utils import run_bass_kernel_spmd

F32 = mybir.dt.float32
BF16 = mybir.dt.bfloat16
I32 = mybir.dt.int32

IN = 128          # input feature dim
OUT = 64          # output feature dim per head
H = 2             # heads
ZC = IN + 2       # z-row: 128 z + 2 s_src (all bf16)
NCORES = 8

N_SRC = 100000
N_DST = 100000
NDST_C = N_DST // NCORES            # 12500 dsts per core
SRC_TILES = 784                     # 784*128 = 100352 >= N_SRC
SRC_PAD = SRC_TILES * 128
SRC_GROUP = 16                      # src tiles per load group (49 groups)
DST_TILES = 98                      # 98*128 = 12544 >= NDST_C
DST_PAD = DST_TILES * 128
DST_GROUP = 7                       # dst tiles per load group (14 groups)
T_WIN = DST_TILES                   # 98 fixed 128-dst windows per core


def _perm_row(n):
    """Z-table row for src node n (layout: contiguous per partition)."""
    g = n // (SRC_GROUP * 128)
    m = n % (SRC_GROUP * 128)
    return g * (SRC_GROUP * 128) + (m % 128) * SRC_GROUP + (m // 128)


def _pack_all(src_idx, dst_idx):
    """Bucket edges into (core, 128-dst window); emit per-window slot
    tensors. Returns eidx [NCORES, T_WIN, 128, 2*KMAX] and K list."""
    core_of = dst_idx // NDST_C
    counts = np.zeros((NCORES, T_WIN), np.int64)
    buckets = [[None] * T_WIN for _ in range(NCORES)]
    for c in range(NCORES):
        m = core_of == c
        s_c = src_idx[m]
        d_c = dst_idx[m] - c * NDST_C
        w_c = d_c // 128
        order = np.argsort(w_c, kind="stable")
        s_c, d_c, w_c = s_c[order], d_c[order], w_c[order]
        starts = np.searchsorted(w_c, np.arange(T_WIN + 1))
        for t in range(T_WIN):
            lo, hi = starts[t], starts[t + 1]
            buckets[c][t] = (s_c[lo:hi], d_c[lo:hi] - t * 128)
            counts[c, t] = hi - lo
    kt = np.maximum(1, -(-counts.max(axis=0) // 128)).astype(np.int64)
    kmax = int(kt.max())
    eidx = np.zeros((NCORES, T_WIN, 128, 2 * kmax), np.int32)
    eidx[:, :, :, kmax:] = -1          # dstrel pad
    rp = _perm_row(np.arange(SRC_PAD))
    GRP = SRC_GROUP * 128
    rmax = np.zeros((T_WIN, kmax), np.int64)
    for c in range(NCORES):
        for t in range(T_WIN):
            s, drel = buckets[c][t]
            g = rp[s]
            o2 = np.argsort(g, kind="stable")   # columns sorted by Z row
            g, drel = g[o2], drel[o2]
            n = len(g)
            e = np.arange(n)
            p, j = e % 128, e // 128
            eidx[c, t, p, j] = g
            eidx[c, t, p, kmax + j] = drel
            for jj in range(kt[t]):
                hi = min((jj + 1) * 128, n)
                if hi > 0:
                    rmax[t, jj] = max(rmax[t, jj], g[min(hi, n) - 1] + 1)
    # round dependency prefix up to whole phase-A write groups
    r_cols = (np.maximum(1, -(-rmax // GRP)) * GRP).clip(max=SRC_PAD)
    return eidx, [int(k) for k in kt], kmax, r_cols.astype(np.int64)


NQ = 4            # SWDGE queues to spread indirect gathers across


def _build_program(K_list, KMAX, R_cols):
    nc = bacc.Bacc("TRN2", target_bir_lowering=False, debug=False,
                   num_devices=NCORES, num_swdge_queues=NQ)
    hsT = nc.dram_tensor("hsrcT", [128, SRC_PAD], BF16, kind="ExternalInput").ap()
    hdT = nc.dram_tensor("hdstT", [128, DST_PAD], BF16, kind="ExternalInput").ap()
    wsr = nc.dram_tensor("wsrc", [128, ZC], BF16, kind="ExternalInput").ap()
    wds = nc.dram_tensor("wdst", [128, 2], BF16, kind="ExternalInput").ap()
    eix = nc.dram_tensor("eidx", [T_WIN, 128, 2 * KMAX], I32,
                         kind="ExternalInput").ap()
    Z = nc.dram_tensor("Z", [SRC_PAD, ZC], BF16, kind="Internal").ap()
    SD = nc.dram_tensor("SD", [DST_PAD, 2], BF16, kind="Internal").ap()
    out = nc.dram_tensor("out", [DST_PAD, OUT], F32, kind="ExternalOutput").ap()

    from concourse.masks import make_identity

    with tile.TileContext(nc) as tc:
        with (
            tc.tile_pool(name="const", bufs=1) as cpool,
            tc.tile_pool(name="pa", bufs=3) as pa_pool,
            tc.tile_pool(name="pz", bufs=3) as pz_pool,
            tc.tile_pool(name="sda", bufs=1) as sd_pool,
            tc.tile_pool(name="ei", bufs=9) as ei_pool,
            tc.tile_pool(name="zg", bufs=9) as zg_pool,
            tc.tile_pool(name="oht", bufs=5) as oht_pool,
            tc.tile_pool(name="wt", bufs=6) as w_pool,
            tc.tile_pool(name="fl", bufs=6) as f_pool,
        ):
            wsrc_t = cpool.tile([128, ZC], BF16)
            nc.sync.dma_start(out=wsrc_t[:], in_=wsr[:, :])
            wdst_t = cpool.tile([128, 2], BF16)
            nc.sync.dma_start(out=wdst_t[:], in_=wds[:, :])
            iota_t = cpool.tile([128, KMAX * 128], I32)
            nc.gpsimd.iota(iota_t[:], [[0, KMAX], [1, 128]], channel_multiplier=0)
            iop_t = cpool.tile([128, 1], BF16)
            nc.gpsimd.iota(iop_t[:], [[0, 1]], channel_multiplier=1,
                           allow_small_or_imprecise_dtypes=True)
            ident_t = cpool.tile([128, 128], BF16)
            make_identity(nc, ident_t[:])

            # ---- Phase A: s_dst first (unblocks phase-B sd machinery) ----
            with (
                tc.tile_pool(name="psA", bufs=4, space="PSUM") as psA_pool,
                tc.tile_pool(name="psD", bufs=2, space="PSUM") as psD_pool,
            ):
                sdall = sd_pool.tile([128, 2 * DST_TILES], BF16)
                for g in range(DST_TILES // DST_GROUP):
                    hTd = pa_pool.tile([128, DST_GROUP * 128], BF16, tag="hTd")
                    nc.scalar.dma_start(
                        out=hTd[:],
                        in_=hdT[:, g * DST_GROUP * 128:(g + 1) * DST_GROUP * 128])
                    for j in range(DST_GROUP):
                        t = g * DST_GROUP + j
                        psd = psD_pool.tile([128, 2], F32, tag="psd")
                        nc.tensor.matmul(
                            out=psd[:], lhsT=hTd[:, j * 128:(j + 1) * 128],
                            rhs=wdst_t[:], start=True, stop=True)
                        nc.any.tensor_copy(
                            out=sdall[:, t * 2:(t + 1) * 2], in_=psd[:])
                nc.sync.dma_start(
                    out=SD[:, :].rearrange("(t p) c -> p t c", p=128),
                    in_=sdall[:].rearrange("p (t c) -> p t c", c=2))

                # ---- Phase A: Z = [z | s_src] bf16 for all src nodes ----
                for g in range(SRC_TILES // SRC_GROUP):
                    hT = pa_pool.tile([128, SRC_GROUP * 128], BF16)
                    nc.scalar.dma_start(
                        out=hT[:],
                        in_=hsT[:, g * SRC_GROUP * 128:(g + 1) * SRC_GROUP * 128])
                    zbig = pz_pool.tile([128, SRC_GROUP * ZC], BF16)
                    for m in range(SRC_GROUP // 2):
                        ps = psA_pool.tile([128, 2 * ZC], F32)
                        for h in range(2):
                            nc.tensor.matmul(
                                out=ps[:, h * ZC:(h + 1) * ZC],
                                lhsT=hT[:, (2 * m + h) * 128:(2 * m + h + 1) * 128],
                                rhs=wsrc_t[:], start=True, stop=True)
                        nc.any.tensor_copy(
                            out=zbig[:, m * 2 * ZC:(m + 1) * 2 * ZC], in_=ps[:])
                    rows = slice(g * SRC_GROUP * 128, (g + 1) * SRC_GROUP * 128)
                    # permuted layout: row g*2048 + p*16 + j <- zbig[p, j]
                    nc.sync.dma_start(
                        out=Z[rows, :].rearrange("(p j) c -> p j c", p=128),
                        in_=zbig[:].rearrange("p (j c) -> p j c", c=ZC))

            # ---- Phase B: fixed 128-dst windows ----
            with (
                tc.tile_pool(name="psB", bufs=4, space="PSUM") as psB_pool,
                tc.tile_pool(name="psOH", bufs=2, space="PSUM") as psOH_pool,
                tc.tile_pool(name="psSD", bufs=2, space="PSUM") as psSD_pool,
            ):
              for t in range(T_WIN):
                  K = K_list[t]
                  NB = (K + 7) // 8          # psOH banks (8 transposes each)
                  ei = ei_pool.tile([128, 2 * KMAX], I32)
                  nc.sync.dma_start(out=ei[:], in_=eix[t, :, :])
                  zg = zg_pool.tile([128, KMAX * ZC], BF16)
                  for j in range(K):
                      # columns are src-sorted: this gather only reads the
                      # Z prefix [0, R) -> overlaps with phase-A tail writes
                      R = int(R_cols[t][j])
                      nc.gpsimd.indirect_dma_start(
                          out=zg[:, j * ZC:(j + 1) * ZC], out_offset=None,
                          in_=Z[0:R, :],
                          in_offset=bass.IndirectOffsetOnAxis(
                              ap=ei[:, j:j + 1], axis=0))
                  zg3 = zg[:].rearrange("p (j c) -> p j c", c=ZC)
                  sdw = w_pool.tile([128, 2], BF16, tag="sdw")
                  nc.scalar.dma_start(out=sdw[:], in_=SD[t * 128:(t + 1) * 128, :])
                  drelF = w_pool.tile([128, KMAX], BF16, tag="drelF")
                  nc.vector.tensor_copy(out=drelF[:, 0:K],
                                        in_=ei[:, KMAX:KMAX + K])
                  # transposed one-hots (slot-dst -> partition), 8 per bank
                  ohs = oht_pool.tile([128, KMAX * 128], BF16, tag="ohs")
                  ps_sd = psSD_pool.tile([128, 2 * KMAX], F32, tag="pssd")
                  for b in range(NB):
                      nb = min(8, K - b * 8)
                      ps_oh = psOH_pool.tile([128, 8 * 128], BF16, tag="psoh")
                      for i in range(nb):
                          j = b * 8 + i
                          nc.tensor.transpose(
                              out=ps_oh[:, i * 128:(i + 1) * 128],
                              in_=drelF[:, j:j + 1].to_broadcast([128, 128]),
                              identity=ident_t[:])
                      nc.vector.tensor_tensor(
                          out=ohs[:, b * 1024:b * 1024 + nb * 128],
                          in0=iop_t[:, 0:1].to_broadcast([128, nb * 128]),
                          in1=ps_oh[:, 0:nb * 128],
                          op=mybir.AluOpType.is_equal)
                  for j in range(K):
                      nc.tensor.matmul(
                          out=ps_sd[:, j * 2:(j + 1) * 2],
                          lhsT=ohs[:, j * 128:(j + 1) * 128],
                          rhs=sdw[:], start=True, stop=True)
                  oht = oht_pool.tile([128, KMAX * 128], BF16)
                  nc.vector.tensor_tensor(
                      out=oht[:, 0:K * 128],
                      in0=ei[:, KMAX:KMAX + K].to_broadcast([128, K, 128]),
                      in1=iota_t[:, 0:K * 128].rearrange(
                          "p (k q) -> p k q", q=128),
                      op=mybir.AluOpType.is_equal)
                  st = w_pool.tile([128, KMAX * 2], F32, tag="st")
                  nc.vector.tensor_tensor(
                      out=st[:, 0:K * 2].rearrange("p (j c) -> p j c", c=2),
                      in0=zg3[:, 0:K, IN:IN + 2],
                      in1=ps_sd[:, 0:K * 2].rearrange("p (j c) -> p j c", c=2),
                      op=mybir.AluOpType.add)
                  st2 = w_pool.tile([128, KMAX * 2], F32, tag="st2")
                  nc.vector.tensor_scalar_mul(out=st2[:, 0:K * 2],
                                              in0=st[:, 0:K * 2], scalar1=0.01)
                  nc.vector.tensor_tensor(out=st[:, 0:K * 2], in0=st[:, 0:K * 2],
                                          in1=st2[:, 0:K * 2],
                                          op=mybir.AluOpType.max)
                  wt = w_pool.tile([128, KMAX * 2], F32, tag="wt")
                  nc.scalar.activation(
                      out=wt[:, 0:K * 2], in_=st[:, 0:K * 2],
                      func=mybir.ActivationFunctionType.Exp)
                  wt3 = wt[:].rearrange("p (j c) -> p j c", c=2)
                  wzb = zg_pool.tile([128, KMAX * ZC], BF16, tag="wzb")
                  wzb3 = wzb[:].rearrange("p (j c) -> p j c", c=ZC)
                  for h in range(H):
                      nc.vector.tensor_tensor(
                          out=wzb3[:, 0:K, h * OUT:(h + 1) * OUT],
                          in0=zg3[:, 0:K, h * OUT:(h + 1) * OUT],
                          in1=wt3[:, 0:K, h:h + 1].to_broadcast([128, K, OUT]),
                          op=mybir.AluOpType.mult)
                  nc.vector.tensor_copy(out=wzb3[:, 0:K, IN:IN + 2],
                                        in_=wt3[:, 0:K, :])
                  ps = psB_pool.tile([128, ZC], F32)
                  for j in range(K):
                      nc.tensor.matmul(
                          out=ps[:],
                          lhsT=oht[:, j * 128:(j + 1) * 128],
                          rhs=wzb[:, j * ZC:(j + 1) * ZC],
                          start=(j == 0), stop=(j == K - 1))
                  den = f_pool.tile([128, 2], F32, tag="den")
                  nc.vector.tensor_scalar_max(
                      out=den[:], in0=ps[:, IN:IN + 2], scalar1=1e-30)
                  rec = f_pool.tile([128, 2], F32, tag="rec")
                  nc.vector.reciprocal(out=rec[:], in_=den[:])
                  o0 = f_pool.tile([128, OUT], F32, tag="o0")
                  nc.vector.tensor_scalar_mul(
                      out=o0[:], in0=ps[:, 0:OUT], scalar1=rec[:, 0:1])
                  ot = f_pool.tile([128, OUT], F32, tag="ot")
                  nc.vector.tensor_scalar_mul(
                      out=ot[:], in0=ps[:, OUT:2 * OUT], scalar1=rec[:, 1:2])
                  nc.vector.tensor_add(out=ot[:], in0=ot[:], in1=o0[:])
                  nc.sync.dma_start(out=out[t * 128:(t + 1) * 128, :], in_=ot[:])

    nc.compile()
    return nc


def _prep_inputs(h_src, h_dst, W_src, W_dst, a_w, src_idx, dst_idx):
    """Host-side sharding/layout prep. Returns in_maps for the 8 cores."""
    hs = np.zeros((SRC_PAD, IN), np.float32)
    hs[:N_SRC] = h_src
    hsrcT = np.ascontiguousarray(hs.T.astype(ml_dtypes.bfloat16))

    # wsrc: [IN, ZC] = [ W[h,o,d] at col h*OUT+o | w~_s ]
    wsr = np.zeros((IN, ZC), np.float32)
    wsr[:, :H * OUT] = W_src.reshape(H * OUT, IN).T
    a_s, a_d = a_w[:, :OUT], a_w[:, OUT:]
    wsr[:, H * OUT:H * OUT + H] = np.einsum("hod,ho->dh", W_src, a_s)
    wsr = wsr.astype(ml_dtypes.bfloat16)
    wds = np.einsum("hod,ho->dh", W_dst, a_d).astype(ml_dtypes.bfloat16)

    eidx, K_list, KMAX, R_cols = _pack_all(src_idx, dst_idx)

    in_maps = []
    for c in range(NCORES):
        hd = np.zeros((DST_PAD, IN), np.float32)
        hd[:NDST_C] = h_dst[c * NDST_C:(c + 1) * NDST_C]
        hdstT = np.ascontiguousarray(hd.T.astype(ml_dtypes.bfloat16))
        in_maps.append({
            "hsrcT": hsrcT,
            "hdstT": hdstT,
            "wsrc": wsr,
            "wdst": wds,
            "eidx": eidx[c],
        })
    return in_maps, K_list, KMAX, R_cols


def _run(inputs, trace=False):
    inputs = {k: np.asarray(v) for k, v in inputs.items()}
    in_maps, K_list, KMAX, R_cols = _prep_inputs(**inputs)
    nc = _build_program(K_list, KMAX, R_cols)
    res = run_bass_kernel_spmd(
        nc, in_maps, core_ids=list(range(NCORES)), trace=trace)
    parts = [res.results[c]["out"][:NDST_C] for c in range(NCORES)]
    return np.concatenate(parts, axis=0), res


def kernel(**inputs):
    out, _ = _run(inputs, trace=False)
    return out
